# revision 4
# baseline (speedup 1.0000x reference)
"""2-layer GCN (GCNConv -> ReLU -> GCNConv -> log_softmax) on 8 TRN2 NeuronCores.

v2: aggregation via bulk dma_gather (InstDMAGatherAnt) instead of per-slot
indirect DMAs. One instruction gathers thousands of 64B rows (elem payload 16
f32, source row stride 256B), smashing the 128-rows-per-994ns SWDGE limit of
indirect_dma_start.

- Nodes sharded by destination across 8 cores; within a shard, nodes are
  degree-sorted so 128-node blocks have near-uniform in-degree.
- Self-loops are appended as ordinary edges (table rows are pre-scaled by
  dis = deg^-1/2, outputs post-scaled by dis).
- int16 gather indices only address <32768 rows, so the table is split into
  4 quarter views; each block's slots are grouped by source quarter (padded
  per (block, quarter) to the cross-core max), and each (chunk-of-blocks,
  quarter) is one dma_gather. Pads point at a per-quarter zero row.
- Per block: 4 strided-view reduce_sums (one per quarter) + 3 adds -> acc.
- Both layers aggregate in 16-feature space; layer-2's linear transform is
  applied after aggregation. Tables exchanged with AllGather (rows padded to
  64 f32 for the 256B-stride requirement).
"""

import numpy as np
import concourse.bacc as bacc
import concourse.bass as bass
import concourse.mybir as mybir
from concourse.tile import TileContext
from concourse.masks import make_identity
from concourse.bass_utils import run_bass_kernel_spmd

F32 = mybir.dt.float32
I16 = mybir.dt.int16

N_NODES = 100000
N_FEAT = 500
HID = 16
N_CLS = 40
N_CORES = 8

QSHIFT = 15  # quarter = row >> 15 (32768 rows per quarter view)
QSIZE = 1 << QSHIFT

# gather mode: "B" = raw 64B payload / 256B stride; "A" = documented 256B
GATHER_MODE = "B"
W_TAB = 64  # table row stride in f32 (256B)
G_CHUNK = 8 if GATHER_MODE == "B" else 3
MAX_GCOLS = 6  # max 128-row columns per dma_gather (768 descriptors,
# probe-verified; single_packet descriptor packet holds <=1024 16B descs)


class _Cfg:
    def __init__(self, n_nodes, fin, hid, ncls, n_cores=8):
        self.N = n_nodes
        self.FIN = fin
        self.H = hid
        self.C = ncls
        self.NC = n_cores
        self.SHARD = n_nodes // n_cores
        self.SHARD_PAD = ((self.SHARD + 127) // 128) * 128
        self.NB = self.SHARD_PAD // 128
        self.TROWS = n_cores * self.SHARD_PAD
        assert self.SHARD_PAD > self.SHARD, "need at least one pad row"
        self.NQ = (self.TROWS + QSIZE - 1) // QSIZE
        assert self.NQ == 4
        self.KC = max(1, (fin + 127) // 128)
        assert fin % self.KC == 0
        self.CHUNK = fin // self.KC
        self.XS = 16  # blocks per xT supertile


def _zero_rows(cfg):
    """Per-quarter zero row (quarter-local index of a known-zero pad row)."""
    zq = []
    for q in range(cfg.NQ):
        lo, hi = q * QSIZE, min((q + 1) * QSIZE, cfg.TROWS)
        found = None
        for c in range(cfg.NC):
            pr = c * cfg.SHARD_PAD + cfg.SHARD  # first pad row of core c
            if lo <= pr < hi:
                found = pr - lo
                break
        assert found is not None, f"no zero row in quarter {q}"
        zq.append(found)
    return zq


def _preprocess(x, edge_index, cfg):
    N, NC, SP, NB = cfg.N, cfg.NC, cfg.SHARD_PAD, cfg.NB
    src = np.asarray(edge_index[0], dtype=np.int64)
    dst = np.asarray(edge_index[1], dtype=np.int64)
    deg = np.bincount(dst, minlength=N).astype(np.int64) + 1
    dis = (1.0 / np.sqrt(deg.astype(np.float64))).astype(np.float32)

    pid = np.empty(N, dtype=np.int64)
    perm_list = []
    for c in range(NC):
        nodes = np.arange(c * cfg.SHARD, (c + 1) * cfg.SHARD)
        order = np.argsort(-deg[nodes], kind="stable")
        local = nodes[order]
        perm_list.append(local)
        pid[local] = c * SP + np.arange(cfg.SHARD)

    # append self-loops as ordinary edges
    loop = np.arange(N, dtype=np.int64)
    src_f = np.concatenate([src, loop])
    dst_f = np.concatenate([dst, loop])
    src_pid = pid[src_f]
    dst_pid = pid[dst_f]

    core_of = dst_pid // SP
    q_of = (src_pid >> QSHIFT).astype(np.int64)

    # per-core edge buckets sorted by (quarter, dst_local); slot ranks within
    cnt_qb = np.zeros((NC, cfg.NQ, SP), dtype=np.int64)
    buckets = []
    for c in range(NC):
        m = core_of == c
        dl = dst_pid[m] - c * SP
        sp_ = src_pid[m]
        qq = q_of[m]
        key = qq * SP + dl
        o = np.argsort(key, kind="stable")
        dl, sp_, qq, key = dl[o], sp_[o], qq[o], key[o]
        cnt = np.bincount(key, minlength=cfg.NQ * SP)
        starts = np.concatenate([[0], np.cumsum(cnt)])[:-1]
        s_arr = np.arange(dl.size) - starts[key]
        buckets.append((dl, sp_, qq, s_arr))
        cnt_qb[c] = cnt.reshape(cfg.NQ, SP)

    # S_qb = max over cores & nodes-in-block of per-(node, quarter) count
    S_qb = cnt_qb.reshape(NC, cfg.NQ, NB, 128).max(axis=(0, 3))  # [NQ, NB]

    # chunk layout
    chunks = []  # list of dicts
    col = 0
    for b0 in range(0, NB, G_CHUNK):
        b1 = min(b0 + G_CHUNK, NB)
        qranges = []  # per q: (colstart, ncols)
        bounds = {}  # (q, b) -> colstart
        c0 = col
        for q in range(cfg.NQ):
            qs = col
            for b in range(b0, b1):
                bounds[(q, b)] = col
                col += int(S_qb[q, b])
            qranges.append((qs, col - qs))
        chunks.append(dict(b0=b0, b1=b1, c0=c0, ncols=col - c0,
                           qranges=qranges, bounds=bounds))
    TOTC = col

    zq = _zero_rows(cfg)
    # per-core column grid of quarter-local int16 indices
    idx16 = np.empty((NC, 128, TOTC), dtype=np.int16)
    colbase = np.zeros((cfg.NQ, NB), dtype=np.int64)
    for ch in chunks:
        for q in range(cfg.NQ):
            for b in range(ch["b0"], ch["b1"]):
                colbase[q, b] = ch["bounds"][(q, b)]
    # fill pads per quarter-range
    for ch in chunks:
        for q in range(cfg.NQ):
            qs, qn = ch["qranges"][q]
            idx16[:, :, qs:qs + qn] = zq[q]
    for c in range(NC):
        dl, sp_, qq, s_arr = buckets[c]
        b_arr = dl // 128
        p_arr = dl % 128
        t_arr = colbase[qq, b_arr] + s_arr
        idx16[c, p_arr, t_arr] = (sp_ - (qq << QSHIFT)).astype(np.int16)

    # wrap: flat i = col*128 + p -> tile[(i%16), i//16], replicated 8x.
    # Per chunk: flat = grid[:, c0:c1].T.flatten()
    idx_wrapped = np.empty((NC, 128, TOTC * 8), dtype=np.int16)
    for c in range(NC):
        pieces = []
        for ch in chunks:
            flat = idx16[c, :, ch["c0"]:ch["c0"] + ch["ncols"]].T.reshape(-1)
            wrap = flat.reshape(-1, 16).T  # [16, ncols*8]
            pieces.append(np.tile(wrap, (8, 1)))
        idx_wrapped[c] = np.concatenate(pieces, axis=1)

    dis_pm = np.zeros((NC, 128, NB), dtype=np.float32)
    for c in range(NC):
        d = np.zeros(SP, dtype=np.float32)
        d[: cfg.SHARD] = dis[perm_list[c]]
        dis_pm[c] = d.reshape(NB, 128).T

    xT = np.zeros((NC, cfg.FIN, SP), dtype=np.float32)
    for c in range(NC):
        xc = np.zeros((SP, cfg.FIN), dtype=np.float32)
        xc[: cfg.SHARD] = x[perm_list[c]]
        xT[c] = np.ascontiguousarray(xc.T)

    return dict(idx_wrapped=idx_wrapped, dis_pm=dis_pm, xT=xT,
                chunks=chunks, TOTC=TOTC, S_qb=S_qb, perm_list=perm_list)


def _raw_dma_gather(nc, out_ap, in_ap, idxs_ap, num_idxs, elem_size,
                    elem_step):
    gp = nc.gpsimd
    stride_bytes = elem_step * mybir.dt.size(in_ap.dtype)
    assert stride_bytes % 256 == 0
    _in_ap = gp.lower_ap_dma(in_ap, for_custom_bir_dma=True)
    _idxs_ap = gp.lower_ap(idxs_ap)
    _out_ap = gp.lower_ap(out_ap)
    return gp.add_instruction(
        mybir.InstDMAGatherAnt(
            name=nc.get_next_instruction_name(),
            ins=[*_in_ap, _idxs_ap,
                 gp.lower_val_access(gp.to_reg(num_idxs))],
            outs=[_out_ap],
            transpose=False,
            num_idxs=num_idxs,
            elem_size=elem_size,
            stride_bytes_256=stride_bytes // 256,
            gen_mode=0,
            single_packet=True,
            queue_num=0,
            sbuf_tokens_per_rank=0,
            sbuf_free_dim_per_rank=0,
            sbuf_free_dim_pad_per_rank=0,
            sbuf_byte_offset=0,
        ))


def _build_kernel(cfg, pre):
    nc = bacc.Bacc("TRN2")
    FIN, H, C, SP, NB = cfg.FIN, cfg.H, cfg.C, cfg.SHARD_PAD, cfg.NB
    KC, CH = cfg.KC, cfg.CHUNK
    chunks, TOTC = pre["chunks"], pre["TOTC"]
    GW = H if GATHER_MODE == "B" else W_TAB

    xT = nc.dram_tensor("xT", [FIN, SP], F32, kind="ExternalInput")
    w1 = nc.dram_tensor("w1", [FIN, H], F32, kind="ExternalInput")
    b1r = nc.dram_tensor("b1r", [128, H], F32, kind="ExternalInput")
    w2 = nc.dram_tensor("w2", [H, C], F32, kind="ExternalInput")
    b2r = nc.dram_tensor("b2r", [128, C], F32, kind="ExternalInput")
    dis_d = nc.dram_tensor("dis", [128, NB], F32, kind="ExternalInput")
    idx_d = nc.dram_tensor("idx", [128, TOTC * 8], I16, kind="ExternalInput")
    out_d = nc.dram_tensor("out", [SP, C], F32, kind="ExternalOutput")

    h1_own = nc.dram_tensor("h1_own", [SP, W_TAB], F32)
    y2_own = nc.dram_tensor("y2_own", [SP, W_TAB], F32)
    table1 = nc.dram_tensor("table1", [cfg.TROWS, W_TAB], F32,
                            addr_space="Shared")
    table2 = nc.dram_tensor("table2", [cfg.TROWS, W_TAB], F32,
                            addr_space="Shared")

    groups = [list(range(cfg.NC))]

    with TileContext(nc) as tc:
        with tc.tile_pool(name="const", bufs=1) as constp, \
             tc.tile_pool(name="xsup", bufs=2) as xsupp, \
             tc.tile_pool(name="ps_h", bufs=4, space="PSUM") as ps_h, \
             tc.tile_pool(name="ps_t", bufs=2, space="PSUM") as ps_t, \
             tc.tile_pool(name="ps_o", bufs=2, space="PSUM") as ps_o, \
             tc.tile_pool(name="hsb", bufs=4) as hsbp, \
             tc.tile_pool(name="g", bufs=2) as gp_pool, \
             tc.tile_pool(name="gi", bufs=2) as gip, \
             tc.tile_pool(name="acc", bufs=4) as accp, \
             tc.tile_pool(name="ep", bufs=4) as epp:

            w1t = constp.tile([CH, KC, H], F32)
            for k in range(KC):
                nc.sync.dma_start(out=w1t[:, k, :],
                                  in_=w1[k * CH:(k + 1) * CH, :])
            w2t = constp.tile([H, C], F32)
            nc.sync.dma_start(out=w2t[:], in_=w2[:])
            b1t = constp.tile([128, H], F32)
            nc.sync.dma_start(out=b1t[:], in_=b1r[:])
            b2t = constp.tile([128, C], F32)
            nc.sync.dma_start(out=b2t[:], in_=b2r[:])
            dis_t = constp.tile([128, NB], F32)
            nc.sync.dma_start(out=dis_t[:], in_=dis_d[:])
            ident = constp.tile([128, 128], F32)
            make_identity(nc, ident[:])

            # Phase A: h1_own = dis * (x @ W1)
            nxs = (NB + cfg.XS - 1) // cfg.XS
            for si in range(nxs):
                b_lo = si * cfg.XS
                b_hi = min(NB, b_lo + cfg.XS)
                w = (b_hi - b_lo) * 128
                xts = xsupp.tile([CH, KC, cfg.XS * 128], F32, tag="xts")
                for k in range(KC):
                    nc.sync.dma_start(
                        out=xts[:, k, :w],
                        in_=xT[k * CH:(k + 1) * CH, b_lo * 128:b_hi * 128])
                for b in range(b_lo, b_hi):
                    j = (b - b_lo) * 128
                    ph = ps_h.tile([128, H], F32, tag="ph")
                    for k in range(KC):
                        nc.tensor.matmul(
                            out=ph[:], lhsT=xts[:, k, j:j + 128],
                            rhs=w1t[:, k, :],
                            start=(k == 0), stop=(k == KC - 1))
                    hsb = hsbp.tile([128, H], F32, tag="hsb")
                    nc.scalar.mul(out=hsb[:], in_=ph[:], mul=dis_t[:, b:b + 1])
                    nc.sync.dma_start(
                        out=h1_own[b * 128:(b + 1) * 128, 0:H], in_=hsb[:])

            nc.gpsimd.collective_compute(
                "AllGather", mybir.AluOpType.bypass, replica_groups=groups,
                ins=[h1_own[:, :]], outs=[table1[:, :]])

            MAXC = max(ch["ncols"] for ch in chunks)

            def aggregate(table, post_block):
                for ch in chunks:
                    ncols = ch["ncols"]
                    g = gp_pool.tile([128, ncols, GW], F32, tag="g",
                                     padded_shape=[128, MAXC, GW])
                    gi = gip.tile([128, ncols * 8], I16, tag="gi",
                                  padded_shape=[128, MAXC * 8])
                    nc.sync.dma_start(
                        out=gi[:, :ncols * 8],
                        in_=idx_d[:, ch["c0"] * 8:(ch["c0"] + ncols) * 8])
                    for q in range(cfg.NQ):
                        qs, qn = ch["qranges"][q]
                        if qn == 0:
                            continue
                        lo = q * QSIZE
                        hi = min(lo + QSIZE, cfg.TROWS)
                        o0 = qs - ch["c0"]
                        # split: SWDGE descriptor scratch holds <16384
                        # descriptors per instruction; stay at <=8192
                        for p0 in range(0, qn, MAX_GCOLS):
                            pn = min(MAX_GCOLS, qn - p0)
                            oo = o0 + p0
                            if GATHER_MODE == "B":
                                _raw_dma_gather(
                                    nc, g[:, oo:oo + pn, :],
                                    table[lo:hi, 0:H],
                                    gi[:, oo * 8:(oo + pn) * 8],
                                    pn * 128, H, W_TAB)
                            else:
                                nc.gpsimd.dma_gather(
                                    out_ap=g[:, oo:oo + pn, :],
                                    in_ap=table[lo:hi, :],
                                    idxs_ap=gi[:, oo * 8:(oo + pn) * 8],
                                    num_idxs=pn * 128,
                                    num_idxs_reg=pn * 128,
                                    elem_size=W_TAB)
                    for b in range(ch["b0"], ch["b1"]):
                        acc = accp.tile([128, H], F32, tag="acc")
                        first = True
                        for q in range(cfg.NQ):
                            S = int(pre["S_qb"][q, b])
                            if S == 0:
                                continue
                            o = ch["bounds"][(q, b)] - ch["c0"]
                            view = g[:, o:o + S, 0:H].transpose([0, 2, 1])
                            if first:
                                nc.vector.reduce_sum(
                                    out=acc[:], in_=view,
                                    axis=mybir.AxisListType.X)
                                first = False
                            else:
                                pq = epp.tile([128, H], F32, tag="pq")
                                nc.vector.reduce_sum(
                                    out=pq[:], in_=view,
                                    axis=mybir.AxisListType.X)
                                nc.vector.tensor_add(out=acc[:], in0=acc[:],
                                                     in1=pq[:])
                        post_block(b, acc)

            def post1(b, acc):
                dis_col = dis_t[:, b:b + 1]
                v = epp.tile([128, H], F32, tag="v1")
                nc.vector.tensor_scalar_mul(out=v[:], in0=acc[:],
                                            scalar1=dis_col)
                nc.vector.tensor_add(out=v[:], in0=v[:], in1=b1t[:])
                r = epp.tile([128, H], F32, tag="r1")
                nc.scalar.activation(out=r[:], in_=v[:],
                                     func=mybir.ActivationFunctionType.Relu)
                y = epp.tile([128, H], F32, tag="y1")
                nc.vector.tensor_scalar_mul(out=y[:], in0=r[:],
                                            scalar1=dis_col)
                nc.sync.dma_start(out=y2_own[b * 128:(b + 1) * 128, 0:H],
                                  in_=y[:])

            aggregate(table1, post1)

            nc.gpsimd.collective_compute(
                "AllGather", mybir.AluOpType.bypass, replica_groups=groups,
                ins=[y2_own[:, :]], outs=[table2[:, :]])

            def post2(b, acc):
                dis_col = dis_t[:, b:b + 1]
                a = epp.tile([128, H], F32, tag="a2")
                nc.vector.tensor_scalar_mul(out=a[:], in0=acc[:],
                                            scalar1=dis_col)
                pt = ps_t.tile([H, 128], F32, tag="pt")
                nc.tensor.transpose(out=pt[:], in_=a[:], identity=ident[:])
                at = epp.tile([H, 128], F32, tag="at")
                nc.vector.tensor_copy(out=at[:], in_=pt[:])
                po = ps_o.tile([128, C], F32, tag="po")
                nc.tensor.matmul(out=po[:], lhsT=at[:], rhs=w2t[:],
                                 start=True, stop=True)
                o1 = epp.tile([128, C], F32, tag="o1")
                nc.vector.tensor_add(out=o1[:], in0=po[:], in1=b2t[:])
                mx = epp.tile([128, 1], F32, tag="mx")
                nc.vector.reduce_max(out=mx[:], in_=o1[:],
                                     axis=mybir.AxisListType.X)
                tt = epp.tile([128, C], F32, tag="tt")
                nc.vector.tensor_scalar(out=tt[:], in0=o1[:], scalar1=mx[:],
                                        scalar2=None,
                                        op0=mybir.AluOpType.subtract)
                ex = epp.tile([128, C], F32, tag="ex")
                nc.scalar.activation(out=ex[:], in_=tt[:],
                                     func=mybir.ActivationFunctionType.Exp)
                sm = epp.tile([128, 1], F32, tag="sm")
                nc.vector.reduce_sum(out=sm[:], in_=ex[:],
                                     axis=mybir.AxisListType.X)
                ls = epp.tile([128, 1], F32, tag="ls")
                nc.scalar.activation(out=ls[:], in_=sm[:],
                                     func=mybir.ActivationFunctionType.Ln)
                fin = epp.tile([128, C], F32, tag="fin")
                nc.vector.tensor_scalar(out=fin[:], in0=tt[:], scalar1=ls[:],
                                        scalar2=None,
                                        op0=mybir.AluOpType.subtract)
                nc.sync.dma_start(out=out_d[b * 128:(b + 1) * 128, :],
                                  in_=fin[:])

            aggregate(table2, post2)

    nc.compile()
    return nc


def kernel(x, edge_index, W1, b1, W2, b2):
    x = np.asarray(x)
    edge_index = np.asarray(edge_index)
    W1 = np.asarray(W1, np.float32)
    b1 = np.asarray(b1, np.float32)
    W2 = np.asarray(W2, np.float32)
    b2 = np.asarray(b2, np.float32)

    cfg = _Cfg(x.shape[0], x.shape[1], W1.shape[1], W2.shape[1], N_CORES)
    pre = _preprocess(x, edge_index, cfg)
    nc = _build_kernel(cfg, pre)

    b1r = np.broadcast_to(b1, (128, cfg.H)).copy()
    b2r = np.broadcast_to(b2, (128, cfg.C)).copy()
    in_maps = []
    for c in range(cfg.NC):
        in_maps.append({
            "xT": pre["xT"][c],
            "w1": W1,
            "b1r": b1r,
            "w2": W2,
            "b2r": b2r,
            "dis": pre["dis_pm"][c],
            "idx": pre["idx_wrapped"][c],
        })
    r = run_bass_kernel_spmd(nc, in_maps, list(range(cfg.NC)))
    out = np.empty((cfg.N, cfg.C), dtype=np.float32)
    for c in range(cfg.NC):
        out[pre["perm_list"][c]] = r.results[c]["out"][: cfg.SHARD]
    return out
